# revision 11
# baseline (speedup 1.0000x reference)
"""Trainium2 Bass kernel for the masked style-attention module.

Shapes (hardcoded): B=4, C_IN=256, C_KEY=448, H=W=64, N=4096.
Sharding: 8 cores = batch (4) x query-row half (2). Each core computes
out[b][:, h*2048:(h+1)*2048] for its (b, h).

Math per core (rows n in its half, all m in 0..4095):
  Fq[c,n] = Wf@ckey + bf      (bias via appended ones-row in contraction)
  G [c,m] = Wg@skey + bg
  Hv[m,c] = (Wh@style + bh)^T (computed directly transposed: lhsT=style)
  S [n,m] = sum_c Fq[c,n] G[c,m] + cm_neg[n]*smi[m]   (mask folded in as an
            extra contraction row; additive -1e15 == masked_fill in fp32)
  P = exp(S)  (no row-max pass: |S| < ~40 so exp never overflows; softmax
            is shift-invariant so result matches the reference)
  mean = (P @ Hv) / rowsum ; m2 = (P @ Hv^2) / rowsum
  out[c,n] = sqrt(relu(m2-mean^2))[n,c]^T * mvn(content)[c,n] + mean[n,c]^T
"""

import numpy as np

import concourse.bass as bass
from concourse import bacc
import concourse.mybir as mybir
import concourse.tile as tile
from concourse.bass_utils import run_bass_kernel_spmd
from concourse.masks import make_identity

AF = mybir.ActivationFunctionType
ALU = mybir.AluOpType
AX = mybir.AxisListType
F32 = mybir.dt.float32
F32R = mybir.dt.float32r

B, C_IN, C_KEY = 4, 256, 448
N = 4096
HALF = 2048
NGRP = 4          # groups of 4 blocks (512 query rows each)
NEG = -1e15
EPS = 1e-5
CORR = N / (N - 1.0)  # unbiased-variance correction for mvn

# contraction tiles over 449 (= C_KEY + bias/mask row)
KT449 = [(0, 128), (128, 128), (256, 128), (384, 65)]
# contraction tiles over 257 (= C_IN + bias row)
KT257 = [(0, 128), (128, 128), (256, 1)]
# output-channel tiles over 448
CO448 = [(0, 128), (128, 128), (256, 128), (384, 64)]


def _r(ap):
    return ap if ap.dtype == F32R else ap.bitcast(F32R)


def _build():
    nc = bacc.Bacc("TRN2", target_bir_lowering=False)

    skey = nc.dram_tensor("skey", [449, N], F32R, kind="ExternalInput")
    wgT = nc.dram_tensor("wgT", [449, 448], F32R, kind="ExternalInput")
    ckey = nc.dram_tensor("ckey", [449, HALF], F32R, kind="ExternalInput")
    wfT = nc.dram_tensor("wfT", [449, 448], F32R, kind="ExternalInput")
    styl = nc.dram_tensor("styl", [257, N], F32R, kind="ExternalInput")
    whT = nc.dram_tensor("whT", [257, 256], F32R, kind="ExternalInput")
    cont = nc.dram_tensor("cont", [256, N], F32, kind="ExternalInput")
    conth = nc.dram_tensor("conth", [256, HALF], F32, kind="ExternalInput")
    smi = nc.dram_tensor("smi", [1, N], F32R, kind="ExternalInput")
    ident_d = nc.dram_tensor("ident", [128, 128], F32R, kind="ExternalInput")
    cmneg = nc.dram_tensor("cmneg", [1, HALF], F32R, kind="ExternalInput")
    out_d = nc.dram_tensor("out", [256, HALF], F32, kind="ExternalOutput")

    with tile.TileContext(nc, pool_alloc_mode="queue") as tc:
        with tc.tile_pool(name="persist", bufs=1) as persist:
            # G_aug: rows 0..447 = Wg@skey+bg, row 448 (g[3][64]) = smi
            g = [persist.tile([128, N], F32R, tag=f"g{i}", name=f"g{i}") for i in range(3)]
            g.append(persist.tile([65, N], F32R, tag="g3", name="g3"))
            # Hv2: [m-tile 128, 32 m-tiles, Hv(256) | Hv^2(256)]
            hv2 = persist.tile([128, 32, 512], F32R, tag="hv2", name="hv2")
            # Wf^T (+bf row) stationary tiles, used every group
            wf_t = persist.tile([128, 4, 448], F32R, tag="wf_t", name="wf_t")
            nc.sync.dma_start(
                wf_t[:, 0:3, :], wfT[0:384, :].rearrange("(k p) c -> p k c", p=128)
            )
            nc.sync.dma_start(wf_t[0:65, 3, :], wfT[384:449, :])
            ident = persist.tile([128, 128], F32R, tag="ident", name="ident")
            nc.sync.dma_start(ident, ident_d[:, :])
            identf = persist.tile([128, 128], F32, tag="identf", name="identf")
            make_identity(nc, identf)
            eps_t = persist.tile([128, 1], F32, tag="eps", name="eps")
            nc.vector.memset(eps_t, EPS)
            a_t = persist.tile([128, 2], F32, tag="a_t", name="a_t")  # mvn scale per ch
            b_t = persist.tile([128, 2], F32, tag="b_t", name="b_t")  # mvn shift per ch

            # ---- Phase A: mvn stats over full content ----
            with tc.tile_pool(name="mvn", bufs=2) as pm:
                for ct in range(2):
                    cx = pm.tile([128, N], F32, tag="cx", name="cx")
                    nc.sync.dma_start(cx, cont[ct * 128 : (ct + 1) * 128, :])
                    stats = pm.tile([128, 8, 6], F32, tag="stats", name="stats")
                    for i in range(8):
                        nc.vector.bn_stats(
                            out=stats[:, i, :], in_=cx[:, i * 512 : (i + 1) * 512]
                        )
                    mv = pm.tile([128, 2], F32, tag="mv", name="mv")
                    nc.vector.bn_aggr(out=mv, in_=stats)
                    sq = pm.tile([128, 1], F32, tag="sq", name="sq")
                    # sqrt(var*CORR + eps)
                    nc.scalar.activation(
                        sq, mv[:, 1:2], AF.Sqrt, bias=eps_t[:, 0:1], scale=CORR
                    )
                    nc.vector.reciprocal(a_t[:, ct : ct + 1], sq)
                    # b = -mean * a
                    nc.vector.scalar_tensor_tensor(
                        out=b_t[:, ct : ct + 1],
                        in0=mv[:, 0:1],
                        scalar=-1.0,
                        in1=a_t[:, ct : ct + 1],
                        op0=ALU.mult,
                        op1=ALU.mult,
                    )

            # ---- Phase B: G projection ----
            with (
                tc.tile_pool(name="projB", bufs=2) as pb,
                tc.tile_pool(name="wgp", bufs=1) as wgp,
                tc.tile_pool(name="psumB", bufs=4, space="PSUM") as ppb,
            ):
                wg_t = wgp.tile([128, 4, 448], F32R, tag="wg_t", name="wg_t")
                nc.sync.dma_start(
                    wg_t[:, 0:3, :],
                    wgT[0:384, :].rearrange("(k p) c -> p k c", p=128),
                )
                nc.sync.dma_start(wg_t[0:65, 3, :], wgT[384:449, :])
                for ch in range(8):
                    sk = pb.tile([128, 4, 512], F32R, tag="sk", name="sk")
                    csl = slice(ch * 512, (ch + 1) * 512)
                    nc.sync.dma_start(
                        sk[:, 0:3, :],
                        skey[0:384, csl].rearrange("(k p) m -> p k m", p=128),
                    )
                    nc.sync.dma_start(sk[0:65, 3, :], skey[384:449, csl])
                    for co, (co0, cosz) in enumerate(CO448):
                        pg = ppb.tile([128, 512], F32, tag="pg", name="pg")
                        for k, (k0, ksz) in enumerate(KT449):
                            nc.tensor.matmul(
                                pg[0:cosz, :],
                                lhsT=_r(wg_t[0:ksz, k, co0 : co0 + cosz]),
                                rhs=_r(sk[0:ksz, k, :]),
                                start=(k == 0),
                                stop=(k == 3),
                            )
                        dst = g[co][0:cosz, ch * 512 : (ch + 1) * 512]
                        if co % 2 == 0:
                            nc.scalar.copy(dst, pg[0:cosz, :])
                        else:
                            nc.vector.tensor_copy(dst, pg[0:cosz, :])
            nc.sync.dma_start(g[3][64:65, :], smi[:, :])

            # ---- Phase C: Hv2 ----
            with (
                tc.tile_pool(name="projC", bufs=1) as pc,
                tc.tile_pool(name="psumC", bufs=4, space="PSUM") as ppc,
            ):
                st01 = pc.tile([128, 2, N], F32R, tag="st01", name="st01")
                nc.sync.dma_start(
                    st01, styl[0:256, :].rearrange("(k p) m -> p k m", p=128)
                )
                st2 = pc.tile([1, N], F32R, tag="st2", name="st2")
                nc.sync.dma_start(st2, styl[256:257, :])
                wh01 = pc.tile([128, 2, 256], F32R, tag="wh01", name="wh01")
                nc.sync.dma_start(
                    wh01, whT[0:256, :].rearrange("(k p) c -> p k c", p=128)
                )
                wh2 = pc.tile([1, 256], F32R, tag="wh2", name="wh2")
                nc.sync.dma_start(wh2, whT[256:257, :])
                for mt in range(32):
                    ph = ppc.tile([128, 256], F32, tag="ph", name="ph")
                    msl = slice(mt * 128, (mt + 1) * 128)
                    for k in range(3):
                        lhsT = st01[:, k, msl] if k < 2 else st2[:, msl]
                        rhs = wh01[:, k, :] if k < 2 else wh2
                        nc.tensor.matmul(
                            ph, lhsT=lhsT, rhs=rhs, start=(k == 0), stop=(k == 2)
                        )
                    nc.vector.tensor_copy(hv2[:, mt, 0:256], ph)
                    nc.scalar.activation(hv2[:, mt, 256:512], ph, AF.Square)

            # ---- Phase D: main attention loop ----
            with (
                tc.tile_pool(name="ckp", bufs=2) as ckp,
                tc.tile_pool(name="fqp", bufs=1) as fqp,
                tc.tile_pool(name="pp", bufs=2) as pp,
                tc.tile_pool(name="ptbp", bufs=1) as ptbp,
                tc.tile_pool(name="fin", bufs=2) as fin,
                tc.tile_pool(name="ppfq", bufs=1, space="PSUM") as ppfq,
                tc.tile_pool(name="pps", bufs=2, space="PSUM") as pps,
                tc.tile_pool(name="ppt", bufs=2, space="PSUM") as ppt,
                tc.tile_pool(name="ppo", bufs=1, space="PSUM") as ppo,
            ):
                for grp in range(NGRP):
                    gsl = slice(grp * 512, (grp + 1) * 512)
                    ck_g = ckp.tile([128, 4, 512], F32R, tag="ck", name="ck")
                    nc.sync.dma_start(
                        ck_g[:, 0:3, :],
                        ckey[0:384, gsl].rearrange("(k p) m -> p k m", p=128),
                    )
                    nc.sync.dma_start(ck_g[0:65, 3, :], ckey[384:449, gsl])
                    fq_g = fqp.tile([128, 4, 512], F32R, tag="fq", name="fq")
                    for cop in range(2):
                        pfq = ppfq.tile([128, 2, 512], F32, tag="pfq", name="pfq")
                        for ci in range(2):
                            co = cop * 2 + ci
                            co0, cosz = CO448[co]
                            for k, (k0, ksz) in enumerate(KT449):
                                nc.tensor.matmul(
                                    pfq[0:cosz, ci, :],
                                    lhsT=_r(wf_t[0:ksz, k, co0 : co0 + cosz]),
                                    rhs=_r(ck_g[0:ksz, k, :]),
                                    start=(k == 0),
                                    stop=(k == 3),
                                )
                            dst = fq_g[0:cosz, co, :]
                            if co % 2 == 0:
                                nc.scalar.copy(dst, pfq[0:cosz, ci, :])
                            else:
                                nc.vector.tensor_copy(dst, pfq[0:cosz, ci, :])
                    nc.sync.dma_start(fq_g[64:65, 3, :], cmneg[:, gsl])

                    for nbi in range(4):
                        nb = grp * 4 + nbi
                        nsl = slice(nbi * 128, (nbi + 1) * 128)
                        rows_t = fin.tile([128, 8], F32, tag="rows", name="rows")
                        po = ppo.tile([128, 512], F32, tag="po", name="po")
                        ptb = ptbp.tile([128, 32, 128], F32R, tag="ptb", name="ptb")
                        for ch in range(8):
                            ps = pps.tile([128, 512], F32, tag="ps", name="ps")
                            for k, (k0, ksz) in enumerate(KT449):
                                nc.tensor.matmul(
                                    ps,
                                    lhsT=_r(fq_g[0:ksz, k, nsl]),
                                    rhs=_r(g[k][0:ksz, ch * 512 : (ch + 1) * 512]),
                                    start=(k == 0),
                                    stop=(k == 3),
                                )
                            pch = pp.tile([128, 512], F32R, tag="pch", name="pch")
                            nc.scalar.activation(
                                pch, ps, AF.Exp, accum_out=rows_t[:, ch : ch + 1]
                            )
                            ptp = ppt.tile([128, 512], F32R, tag="tp", name="tp")
                            for j in range(4):
                                nc.tensor.transpose(
                                    ptp[:, j * 128 : (j + 1) * 128],
                                    pch[:, j * 128 : (j + 1) * 128],
                                    ident,
                                )
                            nc.vector.tensor_copy(
                                ptb[:, ch * 4 : (ch + 1) * 4, :], ptp
                            )
                            for j in range(4):
                                jj = ch * 4 + j
                                nc.tensor.matmul(
                                    po,
                                    lhsT=_r(ptb[:, jj, :]),
                                    rhs=_r(hv2[:, jj, :]),
                                    start=(jj == 0),
                                    stop=(jj == 31),
                                )
                        # finalize block
                        rs = fin.tile([128, 1], F32, tag="rs", name="rs")
                        nc.vector.reduce_sum(rs, rows_t, axis=AX.X)
                        rinv = fin.tile([128, 1], F32, tag="rinv", name="rinv")
                        nc.vector.reciprocal(rinv, rs)
                        mm = fin.tile([128, 512], F32, tag="mm", name="mm")
                        nc.vector.tensor_scalar(mm, po, rinv, None, ALU.mult)
                        msq = fin.tile([128, 256], F32, tag="msq", name="msq")
                        nc.scalar.activation(msq, mm[:, 0:256], AF.Square)
                        var = fin.tile([128, 256], F32, tag="var", name="var")
                        nc.vector.scalar_tensor_tensor(
                            out=var,
                            in0=msq,
                            scalar=-1.0,
                            in1=mm[:, 256:512],
                            op0=ALU.mult,
                            op1=ALU.add,
                        )
                        varr = fin.tile([128, 256], F32, tag="varr", name="varr")
                        nc.vector.tensor_scalar_max(varr, var, 0.0)
                        stdv = fin.tile([128, 256], F32, tag="stdv", name="stdv")
                        nc.scalar.activation(stdv, varr, AF.Sqrt)
                        ptf = ppt.tile([128, 512], F32, tag="tp", name="tp")
                        nc.tensor.transpose(ptf[:, 0:128], mm[:, 0:128], identf)
                        nc.tensor.transpose(ptf[:, 128:256], mm[:, 128:256], identf)
                        nc.tensor.transpose(ptf[:, 256:384], stdv[:, 0:128], identf)
                        nc.tensor.transpose(ptf[:, 384:512], stdv[:, 128:256], identf)
                        ft = fin.tile([128, 512], F32, tag="ft", name="ft")
                        nc.vector.tensor_copy(ft, ptf)
                        for ct in range(2):
                            csl = slice(ct * 128, (ct + 1) * 128)
                            bsl = slice(nb * 128, (nb + 1) * 128)
                            cb = fin.tile([128, 128], F32, tag="cb", name="cb")
                            nc.sync.dma_start(cb, conth[csl, bsl])
                            mvn_t = fin.tile([128, 128], F32, tag="mvn", name="mvn")
                            nc.scalar.activation(
                                mvn_t,
                                cb,
                                AF.Identity,
                                bias=b_t[:, ct : ct + 1],
                                scale=a_t[:, ct : ct + 1],
                            )
                            t1 = fin.tile([128, 128], F32, tag="t1", name="t1")
                            nc.vector.tensor_mul(
                                t1, mvn_t, ft[:, 256 + ct * 128 : 256 + (ct + 1) * 128]
                            )
                            ob = fin.tile([128, 128], F32, tag="ob", name="ob")
                            nc.vector.tensor_add(ob, t1, ft[:, csl])
                            nc.sync.dma_start(out_d[csl, bsl], ob)
    nc.finalize()
    return nc


_nc_cache = None
last_results = None  # BassKernelResults of the most recent run (for test.py)


def prepare_in_maps(
    content,
    style,
    content_key,
    style_key,
    content_mask,
    style_mask,
    Wf,
    bf,
    Wg,
    bg,
    Wh,
    bh,
):
    f32 = np.float32
    ones_n = np.ones((1, N), f32)
    ones_h = np.ones((1, HALF), f32)
    wgT_in = np.ascontiguousarray(
        np.concatenate([np.asarray(Wg, f32).T, np.asarray(bg, f32)[None, :]], 0)
    )
    wfT_in = np.ascontiguousarray(
        np.concatenate([np.asarray(Wf, f32).T, np.asarray(bf, f32)[None, :]], 0)
    )
    whT_in = np.ascontiguousarray(
        np.concatenate([np.asarray(Wh, f32).T, np.asarray(bh, f32)[None, :]], 0)
    )

    in_maps = []
    for c in range(8):
        b, h = divmod(c, 2)
        hsl = slice(h * HALF, (h + 1) * HALF)
        sk = np.asarray(style_key[b], f32).reshape(C_KEY, N)
        ck = np.asarray(content_key[b], f32).reshape(C_KEY, N)[:, hsl]
        st = np.asarray(style[b], f32).reshape(C_IN, N)
        co = np.asarray(content[b], f32).reshape(C_IN, N)
        smi_in = (np.asarray(content_mask, np.int32) * 0).astype(f32)  # placeholder
        smi_in = (np.asarray(style_mask[b], np.int32).reshape(1, N) == 0).astype(f32)
        cm = np.asarray(content_mask[b], np.int32).reshape(N)[hsl]
        cmneg_in = ((cm != 0).astype(f32) * np.float32(NEG))[None, :]
        in_maps.append(
            {
                "skey": np.ascontiguousarray(np.concatenate([sk, ones_n], 0)),
                "wgT": wgT_in,
                "ckey": np.ascontiguousarray(np.concatenate([ck, ones_h], 0)),
                "wfT": wfT_in,
                "styl": np.ascontiguousarray(np.concatenate([st, ones_n], 0)),
                "whT": whT_in,
                "cont": np.ascontiguousarray(co),
                "conth": np.ascontiguousarray(co[:, hsl]),
                "smi": np.ascontiguousarray(smi_in),
                "ident": np.eye(128, dtype=f32),
                "cmneg": np.ascontiguousarray(cmneg_in),
            }
        )

    return in_maps


def get_nc():
    global _nc_cache
    if _nc_cache is None:
        _nc_cache = _build()
    return _nc_cache


def gather_output(outs):
    full = np.empty((B, C_IN, N), np.float32)
    for c in range(8):
        b, h = divmod(c, 2)
        full[b][:, h * HALF : (h + 1) * HALF] = outs[c]
    return full.reshape(B, C_IN, 64, 64)


def kernel(**inputs):
    global last_results
    in_maps = prepare_in_maps(**inputs)
    res = run_bass_kernel_spmd(get_nc(), in_maps, core_ids=list(range(8)))
    last_results = res
    return gather_output([r["out"] for r in res.results])


if __name__ == "__main__":
    rng = np.random.default_rng(0)
    ins = {
        "content": rng.standard_normal((B, C_IN, 64, 64), dtype=np.float32),
        "style": rng.standard_normal((B, C_IN, 64, 64), dtype=np.float32),
        "content_key": rng.standard_normal((B, C_KEY, 64, 64), dtype=np.float32),
        "style_key": rng.standard_normal((B, C_KEY, 64, 64), dtype=np.float32),
        "content_mask": rng.integers(0, 2, (B, 1, 64, 64)).astype(np.int32),
        "style_mask": rng.integers(0, 2, (B, 1, 64, 64)).astype(np.int32),
        "Wf": (rng.standard_normal((C_KEY, C_KEY)) * 0.02).astype(np.float32),
        "bf": (rng.standard_normal((C_KEY,)) * 0.02).astype(np.float32),
        "Wg": (rng.standard_normal((C_KEY, C_KEY)) * 0.02).astype(np.float32),
        "bg": (rng.standard_normal((C_KEY,)) * 0.02).astype(np.float32),
        "Wh": (rng.standard_normal((C_IN, C_IN)) * 0.02).astype(np.float32),
        "bh": (rng.standard_normal((C_IN,)) * 0.02).astype(np.float32),
    }
    out = kernel(**ins)
    print("kernel output", out.shape, out.dtype, np.abs(out).mean())
